# revision 1
# baseline (speedup 1.0000x reference)
"""Trainium2 Bass kernel for nn_DetectTM (nms_detection).

Reference pipeline per row (96 rows of 360000 f32 samples):
  smax   = sliding window-101 max
  med/mad = lower median / MAD over 121 half-overlapping 6000-sample windows
  mad_t  = bilinear upsample of mad to per-sample resolution
  keep   = (x == smax) & (x > 10*mad_t);  out = top_k(x*keep, 100)

Detection requires x > 10*MAD ~ 6.7 sigma, so detections are (provably,
per-dataset) absent or extremely sparse.  The device kernel is a single-pass
*screening* kernel that produces exact per-block order statistics
certificates; the host then proves, per 3000-sample block, that no sample can
pass the threshold — or, for the rare uncertified blocks, resolves them
exactly on tiny slices.

Device work per 3000-sample block b (one SBUF partition):
  cntA = #{x <= -0.1} + 4096 * #{x <= +0.1}     (custom fused DVE op, 1 pass)
  cntB = #{x <= -0.62} + 4096 * #{x <= +0.62}   (custom fused DVE op, 1 pass)
  scrS = sum sign(5.2 - x)                      (ACT engine, 1 pass)

Host certificates (exact counting arguments, sound for any input):
  window w spans blocks w, w+1 (6000 samples); c_w(T) = cb[w](T)+cb[w+1](T)
  med_w in (-0.1, 0.1]   iff  c_w(-0.1) <= 2999 and c_w(0.1) >= 3000
  then #{|x - med_w| <= 0.52} <= c_w(0.62) - c_w(-0.62); if that <= 2999
  the lower median of |x - med_w| (= mad_w) is > 0.52.
  If every window feeding mad_t over block b has mad > 0.52, then
  10*mad_t > 5.2 over the block; scrS == 3000 proves every x < 5.2 strictly;
  hence keep == False over the whole block.
All-false keep means top_k returns scores 0 at indices 0..99 (jax breaks
value ties by lowest index).  Blocks that fail any certificate are resolved
exactly on the host from the raw input (microseconds per block).
"""

import numpy as np

# ---------------------------------------------------------------- constants
N_CORES = 8
ROWS = 96
ROWS_PER_CORE = ROWS // N_CORES        # 12
NT = 360000
BLK = 3000                             # median block / partition stride
NBLK = NT // BLK                       # 120 blocks per row
MED_K = 6000
N_WIN = 121                            # windows per row (incl. reflect tail)
TOP_K = 100
MAXPOOL_K = 101

T_MED = 0.1                            # median bracket half-width
T0 = 0.52                              # certified MAD lower bound
T_MAD = T_MED + T0                     # 0.62 interval count threshold
T_SCREEN = 10.0 * T0                   # 5.2 screening level
PACK_W = 4096.0                        # count packing weight (exact in fp32)
PRE = 1902                             # DVE prefix for the mad-count pass
SUF = BLK - PRE                        # ACT sign-counted suffix elems (898)

BLOCKS_PER_CORE = ROWS_PER_CORE * NBLK       # 1440
TILE_P = 128
N_TILES = (BLOCKS_PER_CORE + TILE_P - 1) // TILE_P   # 12 (last has 32)

_NEG = np.float32(np.finfo(np.float32).min)

# =====================================================================
# Device kernel construction (lazy, cached)
# =====================================================================
_NC_CACHE = {}


def _register_count2():
    """Register the custom fused DVE op COUNT2_ANT:
       out[k] = (in0[k] <= s0) + (in0[k] <= s1)*imm2 ; accum_out = sum(out).
    One DVE pass yields two exact threshold counts (imm2 = 4096)."""
    from operator import add
    import concourse.dve_ops as dve_ops
    from concourse.dve_ops import DveOp
    from concourse.dve_spec import Spec, Src0, C0, C1, C2, Zero, _has_src1, lower
    from concourse.dve_uop import DveOpSpec

    for op in dve_ops.OPS:
        if op.name == "COUNT2_ANT":
            return op

    def _ref(in0, in1, c0, c1, c2):
        out = ((in0 <= c0) + (in0 <= c1) * c2).astype(np.float32)
        return out, out.reshape(out.shape[0], -1).sum(axis=-1, keepdims=True)

    op = DveOp(
        "COUNT2_ANT",
        Spec(body=(Src0 <= C0) + (Src0 <= C1) * C2,
             accum=add, accum_init=Zero, reference=_ref),
        subdim=False,
        uops_sha={},
    )
    dve_ops.OPS.append(op)
    dve_ops.CUSTOM_DVE_SPECS[op.name] = op.spec
    dve_ops._SUB_OPCODE_FOR_NAME[op.name] = (
        dve_ops._CUSTOM_DVE_ROW_BASE + len(dve_ops.OPS) - 1)
    for ver in ("v3",):
        sha = DveOpSpec(
            name=op.name,
            opcode=dve_ops.get_dve_sub_opcode(op.name),
            uops=lower(op.spec, ver=ver),
            rd1_en=_has_src1(op.spec),
        ).sha(ver)
        op.uops_sha[ver] = sha
    return op


def _build_nc():
    import concourse.bacc as bacc
    import concourse.tile as tile
    from concourse import mybir

    count2 = _register_count2()

    nc = bacc.Bacc("TRN2")
    # pre-register ACT bias constants as preamble const APs (no runtime
    # semaphore dependencies).
    for val in (-T_MED, T_MED, -T_MAD, T_MAD, T_SCREEN):
        t = nc.alloc_sbuf_tensor(f"const-f32-{val}", [128, 1], mybir.dt.float32)
        nc.gpsimd.memset(t.ap(), val)
        nc.const_aps.aps[(mybir.dt.float32, val)] = t.ap()
    nc.all_engine_barrier()

    x_in = nc.dram_tensor("x", [BLOCKS_PER_CORE * BLK], mybir.dt.float32,
                          kind="ExternalInput")
    st_out = nc.dram_tensor("stats", [N_TILES, TILE_P, 8], mybir.dt.float32,
                            kind="ExternalOutput")

    with tile.TileContext(nc) as tc:
        with (
            tc.tile_pool(name="xtiles", bufs=4) as xpool,
            tc.tile_pool(name="scr", bufs=2) as scrpool,
            tc.tile_pool(name="scr2", bufs=2) as scr2pool,
            tc.tile_pool(name="stats", bufs=1) as stpool,
        ):
            st = stpool.tile([TILE_P, N_TILES * 8], mybir.dt.float32)
            for tix in range(N_TILES):
                p0 = tix * TILE_P
                pt = min(TILE_P, BLOCKS_PER_CORE - p0)
                xt = xpool.tile([TILE_P, BLK], mybir.dt.float32)
                nc.sync.dma_start(
                    out=xt[:pt],
                    in_=x_in[p0 * BLK:(p0 + pt) * BLK].rearrange(
                        "(p f) -> p f", p=pt))
                scr = scrpool.tile([TILE_P, BLK], mybir.dt.float32)
                scr2 = scr2pool.tile([TILE_P, BLK], mybir.dt.float32)
                c = st[:, tix * 8:(tix + 1) * 8]
                # DVE: med counts over the full block; mad counts over the
                # block prefix (suffix handled by ACT below)
                nc.vector._custom_dve(
                    count2, out=scr[:pt], in0=xt[:pt],
                    s0=-T_MED, s1=T_MED, imm2=PACK_W,
                    accum_out=c[:pt, 0:1])
                nc.vector._custom_dve(
                    count2, out=scr[:pt, :PRE], in0=xt[:pt, :PRE],
                    s0=-T_MAD, s1=T_MAD, imm2=PACK_W,
                    accum_out=c[:pt, 1:2])
                # ACT: full-range screen + mad-threshold sign-sum suffixes
                nc.scalar.activation(
                    out=scr2[:pt], in_=xt[:pt],
                    func=mybir.ActivationFunctionType.Sign,
                    bias=T_SCREEN, scale=-1.0,
                    accum_out=c[:pt, 2:3])
                for col, T in ((5, -T_MAD), (6, T_MAD)):
                    nc.scalar.activation(
                        out=scr2[:pt, PRE:], in_=xt[:pt, PRE:],
                        func=mybir.ActivationFunctionType.Sign,
                        bias=T, scale=-1.0,
                        accum_out=c[:pt, col:col + 1])
            nc.sync.dma_start(
                out=st_out.rearrange("t p c -> p t c"),
                in_=st.rearrange("p (t c) -> p t c", t=N_TILES))
    nc.finalize()
    return nc


def _get_nc():
    if "nc" not in _NC_CACHE:
        _NC_CACHE["nc"] = _build_nc()
    return _NC_CACHE["nc"]


def _run_device(flat):
    """flat: [96, NT] f32 -> stats per core list of [N_TILES, TILE_P, 4]."""
    from concourse.bass_utils import run_bass_kernel_spmd
    nc = _get_nc()
    in_maps = []
    for k in range(N_CORES):
        shard = np.ascontiguousarray(
            flat[k * ROWS_PER_CORE:(k + 1) * ROWS_PER_CORE]).reshape(-1)
        in_maps.append({"x": shard})
    res = run_bass_kernel_spmd(nc, in_maps, core_ids=list(range(N_CORES)))
    return [r["stats"] for r in res.results]


# =====================================================================
# Host-side emulation of the device stats (for testing / fallback)
# =====================================================================
def compute_stats_numpy(flat):
    """Exactly what the device computes, in numpy. flat: [96, NT] f32."""
    out = []
    for k in range(N_CORES):
        shard = flat[k * ROWS_PER_CORE:(k + 1) * ROWS_PER_CORE].reshape(-1)
        st = np.zeros((N_TILES, TILE_P, 8), np.float32)
        blocks = shard.reshape(BLOCKS_PER_CORE, BLK)
        pre, suf = blocks[:, :PRE], blocks[:, PRE:]
        z = np.zeros(BLOCKS_PER_CORE, np.float32)
        cols = [
            (blocks <= np.float32(-T_MED)).sum(1) + PACK_W * (blocks <= np.float32(T_MED)).sum(1),
            (pre <= np.float32(-T_MAD)).sum(1) + PACK_W * (pre <= np.float32(T_MAD)).sum(1),
            np.sign(np.float32(T_SCREEN) - blocks).sum(1),
            z, z,
            np.sign(np.float32(-T_MAD) - suf).sum(1),
            np.sign(np.float32(T_MAD) - suf).sum(1),
        ]
        for tix in range(N_TILES):
            p0 = tix * TILE_P
            pt = min(TILE_P, BLOCKS_PER_CORE - p0)
            for ci, col in enumerate(cols):
                st[tix, :pt, ci] = col[p0:p0 + pt]
        out.append(st)
    return out


# =====================================================================
# Host-side post-processing
# =====================================================================
def _window_slice(xr_padded, w):
    return xr_padded[w * BLK:(w + 2) * BLK]


def _med_mad_window(xr_padded, w, cache):
    got = cache.get(w)
    if got is not None:
        return got
    vals = _window_slice(xr_padded, w)
    mid = (MED_K - 1) // 2
    med = np.partition(vals, mid)[mid]
    mad = np.partition(np.abs(vals - med), mid)[mid]
    cache[w] = (np.float32(med), np.float32(mad))
    return cache[w]


def _resolve_block(xr, xr_padded, b, wcache):
    """Exact keep-mask detections for block b of one row. Returns [(idx,val)]"""
    lo, hi = b * BLK, (b + 1) * BLK
    # sliding 101-max around this block, -inf padded at row edges
    seg = np.full(BLK + 100, _NEG, np.float32)
    s0, s1 = max(lo - 50, 0), min(hi + 50, NT)
    seg[s0 - (lo - 50):s0 - (lo - 50) + (s1 - s0)] = xr[s0:s1]
    from numpy.lib.stride_tricks import sliding_window_view
    smax = sliding_window_view(seg, MAXPOOL_K).max(axis=-1)      # [BLK]

    i = np.arange(lo, hi, dtype=np.float32)
    pos = (i + np.float32(0.5)) / np.float32(BLK) - np.float32(0.5)
    pos = np.maximum(pos, np.float32(0.0))
    x0 = np.minimum(np.floor(pos).astype(np.int32), N_WIN - 1)
    x1 = np.minimum(x0 + 1, N_WIN - 1)
    w = pos - x0.astype(np.float32)
    mad_by_w = np.zeros(N_WIN, np.float32)
    for ww in np.unique(np.concatenate([x0, x1])):
        mad_by_w[ww] = _med_mad_window(xr_padded, int(ww), wcache)[1]
    mad0 = mad_by_w[x0]
    mad1 = mad_by_w[x1]
    mad_t = (mad0 * (np.float32(1.0) - w) + mad1 * w).astype(np.float32)

    xb = xr[lo:hi]
    keep = (xb == smax) & (xb > np.float32(10.0) * mad_t)
    idx = np.nonzero(keep)[0]
    return [(int(lo + j), np.float32(xb[j])) for j in idx]


def _zero_fill_indices(xr, det_pos, k):
    """Lowest k indices of +0.0 entries of masked = x*keep.

    jax.lax.top_k uses the IEEE total order, so +0.0 (x >= 0, keep False)
    ranks above -0.0 (x < 0); ties break by lowest index."""
    scan = 1024
    while True:
        idx = np.nonzero(~np.signbit(xr[:scan]))[0]
        if det_pos:
            idx = idx[~np.isin(idx, list(det_pos))]
        if len(idx) >= k or scan >= NT:
            break
        scan *= 8
    if len(idx) >= k:
        return idx[:k].tolist()
    fills = idx.tolist()           # pathological: < k non-negatives in row
    j = 0
    have = set(fills)
    while len(fills) < k:
        if j not in det_pos and j not in have:
            fills.append(j)
        j += 1
    return fills


def _assemble_row(xr, dets):
    """jax.lax.top_k(masked, 100) given the exact sparse detection list."""
    if not dets:
        fills = _zero_fill_indices(xr, (), TOP_K)
        return (np.array([xr[j] * np.float32(0.0) for j in fills], np.float32),
                np.array(fills, np.int32))
    dets = sorted(dets, key=lambda t: (-t[1], t[0]))
    if len(dets) >= TOP_K:
        top = dets[:TOP_K]
        return (np.array([v for _, v in top], np.float32),
                np.array([i for i, _ in top], np.int32))
    det_pos = set(i for i, _ in dets)
    fills = _zero_fill_indices(xr, det_pos, TOP_K - len(dets))
    vals = [v for _, v in dets] + [np.float32(xr[j] * np.float32(0.0)) for j in fills]
    idxs = [i for i, _ in dets] + fills
    return np.array(vals, np.float32), np.array(idxs, np.int32)


def host_postprocess(flat, stats_list):
    """flat: [96, NT] f32; stats_list: per-core [N_TILES, TILE_P, 4].
    Returns (scores [96,100] f32, inds [96,100] i32)."""
    # ---- decode per-block counts -------------------------------------
    # estimated counts: exact DVE prefix + ACT suffix sign-sum (each exact
    # float tie at a threshold shifts the estimate by 1/2; certs carry slack)
    cA_lo = np.zeros((ROWS, NBLK + 2), np.float64)
    cA_hi = np.zeros((ROWS, NBLK + 2), np.float64)
    cB_lo = np.zeros((ROWS, NBLK + 2), np.float64)
    cB_hi = np.zeros((ROWS, NBLK + 2), np.float64)
    screen_ok = np.zeros((ROWS, NBLK), bool)

    g = np.arange(BLOCKS_PER_CORE)
    t_of_g, p_of_g = divmod(g, TILE_P)
    r_of_g, b_of_g = divmod(g, NBLK)
    for k in range(N_CORES):
        st = np.asarray(stats_list[k], np.float64)
        a = st[t_of_g, p_of_g, 0]
        bb = st[t_of_g, p_of_g, 1]
        s = st[t_of_g, p_of_g, 2]
        rows = k * ROWS_PER_CORE + r_of_g
        hiA = np.floor(a / PACK_W)
        hiB = np.floor(bb / PACK_W)
        cA_hi[rows, b_of_g] = hiA
        cA_lo[rows, b_of_g] = a - PACK_W * hiA
        cB_hi[rows, b_of_g] = hiB + (SUF + st[t_of_g, p_of_g, 6]) / 2.0
        cB_lo[rows, b_of_g] = (bb - PACK_W * hiB) + (SUF + st[t_of_g, p_of_g, 5]) / 2.0
        screen_ok[rows, b_of_g] = (s == 3000.0)

    # ---- reflect-tail blocks 120, 121 (host-side exact counts) -------
    # padded[360000+k] = x[359998-k]; block 120 = x[356999:359999],
    # block 121 = x[353999:356999] as multisets.
    for bidx, sl in ((NBLK, slice(356999, 359999)), (NBLK + 1, slice(353999, 356999))):
        seg = flat[:, sl]
        cA_lo[:, bidx] = (seg <= np.float32(-T_MED)).sum(1)
        cA_hi[:, bidx] = (seg <= np.float32(T_MED)).sum(1)
        cB_lo[:, bidx] = (seg <= np.float32(-T_MAD)).sum(1)
        cB_hi[:, bidx] = (seg <= np.float32(T_MAD)).sum(1)

    # ---- window certificates -----------------------------------------
    w = np.arange(N_WIN)
    cwA_lo = cA_lo[:, w] + cA_lo[:, w + 1]
    cwA_hi = cA_hi[:, w] + cA_hi[:, w + 1]
    cwB_lo = cB_lo[:, w] + cB_lo[:, w + 1]
    cwB_hi = cB_hi[:, w] + cB_hi[:, w + 1]
    # med counts are exact; mad slack 32 absorbs suffix sign-sum ties
    med_ok = (cwA_lo <= 2999) & (cwA_hi >= 3000)
    mad_ok = med_ok & ((cwB_hi - cwB_lo) <= 2999 - 32)   # => mad_w > T0

    # block b is clear if screen passed and every window feeding its mad_t
    # interpolation (b-1, b, b+1 clamped to [0, 120]) certifies mad > T0.
    win_ok_ext = np.ones((ROWS, N_WIN + 2), bool)
    win_ok_ext[:, 1:N_WIN + 1] = mad_ok
    b = np.arange(NBLK)
    wlo = np.maximum(b - 1, 0)
    whi = np.minimum(b + 1, N_WIN - 1)
    blocks_ok = (screen_ok
                 & win_ok_ext[:, wlo + 1] & win_ok_ext[:, b + 1]
                 & win_ok_ext[:, whi + 1])

    # ---- exact resolution of unclear blocks --------------------------
    # clear rows: no detections -> scores all +0.0 at the first 100
    # non-negative positions (total-order tie-break, see _zero_fill_indices)
    scores = np.zeros((ROWS, TOP_K), np.float32)
    inds = np.empty((ROWS, TOP_K), np.int32)
    for r in range(ROWS):
        inds[r] = _zero_fill_indices(flat[r], (), TOP_K)
    bad_rows = np.nonzero(~blocks_ok.all(axis=1))[0]
    for r in bad_rows:
        xr = flat[r]
        xr_padded = np.pad(xr, (0, MED_K), mode="reflect")
        wcache = {}
        dets = []
        for bb in np.nonzero(~blocks_ok[r])[0]:
            dets.extend(_resolve_block(xr, xr_padded, int(bb), wcache))
        s, i = _assemble_row(xr, dets)
        scores[r] = s
        inds[r] = i
    return scores, inds


# =====================================================================
# Entry point
# =====================================================================
def _spot_check(flat, stats_list, n_checks=12):
    """Verify device counts on a few random blocks; True iff all exact."""
    rng = np.random.default_rng(0)
    for _ in range(n_checks):
        k = int(rng.integers(N_CORES))
        g = int(rng.integers(BLOCKS_PER_CORE))
        tix, p = divmod(g, TILE_P)
        seg = flat[k * ROWS_PER_CORE:(k + 1) * ROWS_PER_CORE].reshape(-1)[
            g * BLK:(g + 1) * BLK]
        a = ((seg <= np.float32(-T_MED)).sum()
             + PACK_W * (seg <= np.float32(T_MED)).sum())
        b = ((seg[:PRE] <= np.float32(-T_MAD)).sum()
             + PACK_W * (seg[:PRE] <= np.float32(T_MAD)).sum())
        s = np.sign(np.float32(T_SCREEN) - seg).sum()
        s5 = np.sign(np.float32(-T_MAD) - seg[PRE:]).sum()
        st = np.asarray(stats_list[k])
        if not (st[tix, p, 0] == a and st[tix, p, 1] == b
                and st[tix, p, 2] == s and st[tix, p, 5] == s5):
            return False
    return True


def kernel(xcorr: np.ndarray):
    flat = np.ascontiguousarray(xcorr, dtype=np.float32).reshape(ROWS, NT)
    try:
        stats_list = _run_device(flat)
        if not _spot_check(flat, stats_list):
            stats_list = compute_stats_numpy(flat)
    except Exception:
        # device unavailable / run failed: exact host fallback
        stats_list = compute_stats_numpy(flat)
    scores, inds = host_postprocess(flat, stats_list)
    return (scores.reshape(2, 3, 16, TOP_K),
            inds.reshape(2, 3, 16, TOP_K).astype(np.int32))



# revision 2
# speedup vs baseline: 1.1789x; 1.1789x over previous
"""Trainium2 Bass kernel for nn_DetectTM (nms_detection).

Reference pipeline per row (96 rows of 360000 f32 samples):
  smax   = sliding window-101 max
  med/mad = lower median / MAD over 121 half-overlapping 6000-sample windows
  mad_t  = bilinear upsample of mad to per-sample resolution
  keep   = (x == smax) & (x > 10*mad_t);  out = top_k(x*keep, 100)

Detection requires x > 10*MAD ~ 6.7 sigma, so detections are (per-dataset)
absent or extremely sparse.  The device kernel is a single-pass *screening*
kernel producing per-block order-statistic certificates; the host proves,
per 3000-sample block, that no sample can pass the threshold — or resolves
the rare uncertified blocks exactly on tiny slices.

Device work per 3000-sample block b (one SBUF partition row):
  Custom 2-stream DVE ops stream TWO elements per cycle (Src0+Src1 on the
  two SBUF read ports), with packed dual counts per accumulator:
    c0 = #{x[0:2*PA] <= -0.1} + 4096 * #{x[0:PA]    >= 5.2}   (op CA2S0)
    c1 = #{x[0:2*PA] <= +0.1} + 4096 * #{x[PA:2*PA] >= 5.2}   (op CA2S1)
    c2 = #{fl32(x*x) <= fl32(0.62^2)} over the whole block     (op CB2)
  ACT covers the block suffix [2*PA, 3000) with three Sign accumulations:
    c3 = sum sign(-0.1 - x),  c4 = sum sign(0.1 - x),  c5 = sum sign(5.2 - x)

Host certificates (counting arguments, sound for any input up to a tie
slack identical in kind to the sign-tie slack of the original design):
  window w spans blocks w, w+1 (6000 samples)
  med_w in (-0.1, 0.1]  if  sum of #{x<=-0.1} upper bounds <= 2999 - slack
                        and sum of #{x<=+0.1} lower bounds >= 3000
  then #{|x - med_w| <= 0.52} <= #{x in [-0.62, 0.62]} <= c2[w]+c2[w+1]
  (monotone RNE rounding: |x| <= 0.62  ==>  fl32(x*x) <= fl32(0.62^2));
  if that is <= 2999 the lower median of |x - med_w| (= mad_w) is > 0.52.
  If every window feeding mad_t over block b certifies mad > 0.52, then
  10*mad_t > 5.2 over the block; zero screen counts + full suffix sign-sum
  prove every x < 5.2; hence keep == False over the whole block.
All-false keep means top_k returns scores 0 at indices 0..99 (jax breaks
value ties by lowest index).  Blocks failing any certificate are resolved
exactly on the host from the raw input (microseconds per block).
"""

import numpy as np

# ---------------------------------------------------------------- constants
N_CORES = 8
ROWS = 96
ROWS_PER_CORE = ROWS // N_CORES        # 12
NT = 360000
BLK = 3000                             # median block / partition stride
NBLK = NT // BLK                       # 120 blocks per row
MED_K = 6000
N_WIN = 121                            # windows per row (incl. reflect tail)
TOP_K = 100
MAXPOOL_K = 101

T_A = 0.1                              # median bracket half-width
T0 = 0.52                              # certified MAD lower bound
T_B = 0.62                             # B-interval threshold (0.1 + 0.52)
T2B = float(np.float32(T_B) * np.float32(T_B))   # squared, f32-exact
T_S = 10.0 * T0                        # 5.2 screening level
PACK_W = 4096.0                        # count packing weight (exact in fp32)
PA = 896                               # per-half A-op extent (DVE/ACT balance)
NSUF = BLK - 2 * PA                    # ACT suffix elems (1208)
HB = BLK // 2                          # CB2 half extent (1500)
TIE_SLACK = 32                         # sign-tie budget on the -0.1 cert

BLOCKS_PER_CORE = ROWS_PER_CORE * NBLK       # 1440
TILE_P = 128
N_TILES = (BLOCKS_PER_CORE + TILE_P - 1) // TILE_P   # 12 (last has 32)
ST_C = 8                               # stats columns per block

_NEG = np.float32(np.finfo(np.float32).min)

# =====================================================================
# Device kernel construction (lazy, cached)
# =====================================================================
_NC_CACHE = {}


def _register_ops():
    """Register the three custom fused 2-stream DVE counting ops.

    Each streams Src0 and Src1 (both SBUF read ports) at 1 elem/cycle per
    port — 2 data elements per cycle — and folds the body into one running
    f32 accumulator (exact: all partial sums < 2^24).
      CA2S0: (in0<=c0) + (in1<=c0) + (in0>=c1)*c2 ; accum add
      CA2S1: (in0<=c0) + (in1<=c0) + (in1>=c1)*c2 ; accum add
      CB2:   (in0*in0<=c0) + (in1*in1<=c0)        ; accum add
    """
    from operator import add
    import concourse.dve_ops as dve_ops
    from concourse.dve_ops import DveOp
    from concourse.dve_spec import (
        Spec, Src0, Src1, C0, C1, C2, Zero, sq, _has_src1, lower)
    from concourse.dve_uop import DveOpSpec

    def _ref_ca2s0(in0, in1, c0, c1, c2):
        out = ((in0 <= c0) + (in1 <= c0) + (in0 >= c1) * c2).astype(np.float32)
        return out, out.reshape(out.shape[0], -1).sum(axis=-1, keepdims=True)

    def _ref_ca2s1(in0, in1, c0, c1, c2):
        out = ((in0 <= c0) + (in1 <= c0) + (in1 >= c1) * c2).astype(np.float32)
        return out, out.reshape(out.shape[0], -1).sum(axis=-1, keepdims=True)

    def _ref_cb2(in0, in1, c0, c1, c2):
        out = ((in0 * in0 <= c0) + (in1 * in1 <= c0)).astype(np.float32)
        return out, out.reshape(out.shape[0], -1).sum(axis=-1, keepdims=True)

    defs = [
        ("CA2S0_ANT",
         ((Src0 <= C0) + (Src1 <= C0)) + (Src0 >= C1) * C2, _ref_ca2s0),
        ("CA2S1_ANT",
         ((Src0 <= C0) + (Src1 <= C0)) + (Src1 >= C1) * C2, _ref_ca2s1),
        ("CB2_ANT",
         (sq(Src0) <= C0) + (sq(Src1) <= C0), _ref_cb2),
    ]
    have = {op.name: op for op in dve_ops.OPS}
    out = []
    for name, body, ref in defs:
        if name in have:
            out.append(have[name])
            continue
        op = DveOp(
            name,
            Spec(body=body, accum=add, accum_init=Zero, reference=ref),
            subdim=False,
            uops_sha={},
        )
        dve_ops.OPS.append(op)
        dve_ops.CUSTOM_DVE_SPECS[op.name] = op.spec
        dve_ops._SUB_OPCODE_FOR_NAME[op.name] = (
            dve_ops._CUSTOM_DVE_ROW_BASE + len(dve_ops.OPS) - 1)
        for ver in ("v3",):
            sha = DveOpSpec(
                name=op.name,
                opcode=dve_ops.get_dve_sub_opcode(op.name),
                uops=lower(op.spec, ver=ver),
                rd1_en=_has_src1(op.spec),
            ).sha(ver)
            op.uops_sha[ver] = sha
        out.append(op)
    return out


def _build_nc():
    import concourse.bacc as bacc
    import concourse.tile as tile
    from concourse import mybir

    ca2s0, ca2s1, cb2 = _register_ops()

    nc = bacc.Bacc("TRN2")
    # pre-register ACT bias constants as preamble const APs (no runtime
    # semaphore dependencies).
    for val in (-T_A, T_A, T_S):
        t = nc.alloc_sbuf_tensor(f"const-f32-{val}", [128, 1], mybir.dt.float32)
        nc.gpsimd.memset(t.ap(), val)
        nc.const_aps.aps[(mybir.dt.float32, val)] = t.ap()
    nc.all_engine_barrier()

    x_in = nc.dram_tensor("x", [BLOCKS_PER_CORE * BLK], mybir.dt.float32,
                          kind="ExternalInput")
    st_out = nc.dram_tensor("stats", [N_TILES, TILE_P, ST_C], mybir.dt.float32,
                            kind="ExternalOutput")

    with tile.TileContext(nc) as tc:
        with (
            tc.tile_pool(name="xtiles", bufs=4) as xpool,
            tc.tile_pool(name="scr", bufs=2) as scrpool,
            tc.tile_pool(name="scr2", bufs=2) as scr2pool,
            tc.tile_pool(name="stats", bufs=1) as stpool,
        ):
            st = stpool.tile([TILE_P, N_TILES * ST_C], mybir.dt.float32)
            for tix in range(N_TILES):
                p0 = tix * TILE_P
                pt = min(TILE_P, BLOCKS_PER_CORE - p0)
                xt = xpool.tile([TILE_P, BLK], mybir.dt.float32)
                nc.sync.dma_start(
                    out=xt[:pt],
                    in_=x_in[p0 * BLK:(p0 + pt) * BLK].rearrange(
                        "(p f) -> p f", p=pt))
                scr = scrpool.tile([TILE_P, 2 * PA + HB], mybir.dt.float32)
                scr2 = scr2pool.tile([TILE_P, 3 * NSUF], mybir.dt.float32)
                c = st[:, tix * ST_C:(tix + 1) * ST_C]
                # DVE: 2-stream packed counting over [0, 2*PA) + full-block B
                nc.vector._custom_dve(
                    ca2s0, out=scr[:pt, :PA],
                    in0=xt[:pt, :PA], in1=xt[:pt, PA:2 * PA],
                    s0=-T_A, s1=T_S, imm2=PACK_W,
                    accum_out=c[:pt, 0:1])
                nc.vector._custom_dve(
                    ca2s1, out=scr[:pt, PA:2 * PA],
                    in0=xt[:pt, :PA], in1=xt[:pt, PA:2 * PA],
                    s0=T_A, s1=T_S, imm2=PACK_W,
                    accum_out=c[:pt, 1:2])
                nc.vector._custom_dve(
                    cb2, out=scr[:pt, 2 * PA:],
                    in0=xt[:pt, :HB], in1=xt[:pt, HB:],
                    s0=T2B,
                    accum_out=c[:pt, 2:3])
                # ACT: sign-sum suffixes for both A thresholds + screen
                for col, bias in ((3, -T_A), (4, T_A), (5, T_S)):
                    nc.scalar.activation(
                        out=scr2[:pt, (col - 3) * NSUF:(col - 2) * NSUF],
                        in_=xt[:pt, 2 * PA:],
                        func=mybir.ActivationFunctionType.Sign,
                        bias=bias, scale=-1.0,
                        accum_out=c[:pt, col:col + 1])
            nc.sync.dma_start(
                out=st_out.rearrange("t p c -> p t c"),
                in_=st.rearrange("p (t c) -> p t c", t=N_TILES))
    nc.finalize()
    return nc


def _get_nc():
    if "nc" not in _NC_CACHE:
        _NC_CACHE["nc"] = _build_nc()
    return _NC_CACHE["nc"]


def _run_device(flat):
    """flat: [96, NT] f32 -> per-core list of stats [N_TILES, TILE_P, ST_C]."""
    from concourse.bass_utils import run_bass_kernel_spmd
    nc = _get_nc()
    in_maps = []
    for k in range(N_CORES):
        shard = np.ascontiguousarray(
            flat[k * ROWS_PER_CORE:(k + 1) * ROWS_PER_CORE]).reshape(-1)
        in_maps.append({"x": shard})
    res = run_bass_kernel_spmd(nc, in_maps, core_ids=list(range(N_CORES)))
    return [r["stats"] for r in res.results]


# =====================================================================
# Host-side emulation of the device stats (for testing / fallback)
# =====================================================================
def _block_stats(blocks):
    """Device per-block stats for blocks: [n, BLK] f32 -> [n, ST_C] f32."""
    h0, h1 = blocks[:, :PA], blocks[:, PA:2 * PA]
    suf = blocks[:, 2 * PA:]
    q0, q1 = blocks[:, :HB], blocks[:, HB:]
    n = blocks.shape[0]
    st = np.zeros((n, ST_C), np.float32)
    st[:, 0] = ((h0 <= np.float32(-T_A)).sum(1) + (h1 <= np.float32(-T_A)).sum(1)
                + PACK_W * (h0 >= np.float32(T_S)).sum(1))
    st[:, 1] = ((h0 <= np.float32(T_A)).sum(1) + (h1 <= np.float32(T_A)).sum(1)
                + PACK_W * (h1 >= np.float32(T_S)).sum(1))
    st[:, 2] = ((q0 * q0 <= np.float32(T2B)).sum(1)
                + (q1 * q1 <= np.float32(T2B)).sum(1))
    st[:, 3] = np.sign(np.float32(-T_A) - suf).sum(1)
    st[:, 4] = np.sign(np.float32(T_A) - suf).sum(1)
    st[:, 5] = np.sign(np.float32(T_S) - suf).sum(1)
    return st


def compute_stats_numpy(flat):
    """Exactly what the device computes, in numpy. flat: [96, NT] f32."""
    out = []
    for k in range(N_CORES):
        shard = flat[k * ROWS_PER_CORE:(k + 1) * ROWS_PER_CORE].reshape(-1)
        st = np.zeros((N_TILES, TILE_P, ST_C), np.float32)
        cols = _block_stats(shard.reshape(BLOCKS_PER_CORE, BLK))
        for tix in range(N_TILES):
            p0 = tix * TILE_P
            pt = min(TILE_P, BLOCKS_PER_CORE - p0)
            st[tix, :pt] = cols[p0:p0 + pt]
        out.append(st)
    return out


# =====================================================================
# Host-side post-processing
# =====================================================================
def _window_slice(xr_padded, w):
    return xr_padded[w * BLK:(w + 2) * BLK]


def _med_mad_window(xr_padded, w, cache):
    got = cache.get(w)
    if got is not None:
        return got
    vals = _window_slice(xr_padded, w)
    mid = (MED_K - 1) // 2
    med = np.partition(vals, mid)[mid]
    mad = np.partition(np.abs(vals - med), mid)[mid]
    cache[w] = (np.float32(med), np.float32(mad))
    return cache[w]


def _resolve_block(xr, xr_padded, b, wcache):
    """Exact keep-mask detections for block b of one row. Returns [(idx,val)]"""
    lo, hi = b * BLK, (b + 1) * BLK
    # sliding 101-max around this block, -inf padded at row edges
    seg = np.full(BLK + 100, _NEG, np.float32)
    s0, s1 = max(lo - 50, 0), min(hi + 50, NT)
    seg[s0 - (lo - 50):s0 - (lo - 50) + (s1 - s0)] = xr[s0:s1]
    from numpy.lib.stride_tricks import sliding_window_view
    smax = sliding_window_view(seg, MAXPOOL_K).max(axis=-1)      # [BLK]

    i = np.arange(lo, hi, dtype=np.float32)
    pos = (i + np.float32(0.5)) / np.float32(BLK) - np.float32(0.5)
    pos = np.maximum(pos, np.float32(0.0))
    x0 = np.minimum(np.floor(pos).astype(np.int32), N_WIN - 1)
    x1 = np.minimum(x0 + 1, N_WIN - 1)
    w = pos - x0.astype(np.float32)
    mad_by_w = np.zeros(N_WIN, np.float32)
    for ww in np.unique(np.concatenate([x0, x1])):
        mad_by_w[ww] = _med_mad_window(xr_padded, int(ww), wcache)[1]
    mad0 = mad_by_w[x0]
    mad1 = mad_by_w[x1]
    mad_t = (mad0 * (np.float32(1.0) - w) + mad1 * w).astype(np.float32)

    xb = xr[lo:hi]
    keep = (xb == smax) & (xb > np.float32(10.0) * mad_t)
    idx = np.nonzero(keep)[0]
    return [(int(lo + j), np.float32(xb[j])) for j in idx]


def _zero_fill_indices(xr, det_pos, k):
    """Lowest k indices of +0.0 entries of masked = x*keep.

    jax.lax.top_k uses the IEEE total order, so +0.0 (x >= 0, keep False)
    ranks above -0.0 (x < 0); ties break by lowest index."""
    scan = 1024
    while True:
        idx = np.nonzero(~np.signbit(xr[:scan]))[0]
        if det_pos:
            idx = idx[~np.isin(idx, list(det_pos))]
        if len(idx) >= k or scan >= NT:
            break
        scan *= 8
    if len(idx) >= k:
        return idx[:k].tolist()
    fills = idx.tolist()           # pathological: < k non-negatives in row
    j = 0
    have = set(fills)
    while len(fills) < k:
        if j not in det_pos and j not in have:
            fills.append(j)
        j += 1
    return fills


def _assemble_row(xr, dets):
    """jax.lax.top_k(masked, 100) given the exact sparse detection list."""
    if not dets:
        fills = _zero_fill_indices(xr, (), TOP_K)
        return (np.array([xr[j] * np.float32(0.0) for j in fills], np.float32),
                np.array(fills, np.int32))
    dets = sorted(dets, key=lambda t: (-t[1], t[0]))
    if len(dets) >= TOP_K:
        top = dets[:TOP_K]
        return (np.array([v for _, v in top], np.float32),
                np.array([i for i, _ in top], np.int32))
    det_pos = set(i for i, _ in dets)
    fills = _zero_fill_indices(xr, det_pos, TOP_K - len(dets))
    vals = [v for _, v in dets] + [np.float32(xr[j] * np.float32(0.0)) for j in fills]
    idxs = [i for i, _ in dets] + fills
    return np.array(vals, np.float32), np.array(idxs, np.int32)


def host_postprocess(flat, stats_list):
    """flat: [96, NT] f32; stats_list: per-core [N_TILES, TILE_P, ST_C].
    Returns (scores [96,100] f32, inds [96,100] i32)."""
    # ---- decode per-block certificate quantities ---------------------
    # up_lo: upper bound of #{x <= -0.1}  (DVE prefix exact + sign suffix;
    #        the sign estimate errs by 1/2 per exact f32 tie — TIE_SLACK
    #        at the window level absorbs it, as in the original design)
    # dn_hi: lower bound of #{x <= +0.1}  (ties only increase the count)
    # n_b:   exact #{fl32(x^2) <= fl32(0.62^2)}  (superset of |x| <= 0.62)
    # screen_ok: certified no x >= 5.2 in the block
    up_lo = np.zeros((ROWS, NBLK + 2), np.float64)
    dn_hi = np.zeros((ROWS, NBLK + 2), np.float64)
    n_b = np.zeros((ROWS, NBLK + 2), np.float64)
    screen_ok = np.zeros((ROWS, NBLK), bool)

    g = np.arange(BLOCKS_PER_CORE)
    t_of_g, p_of_g = divmod(g, TILE_P)
    r_of_g, b_of_g = divmod(g, NBLK)
    for k in range(N_CORES):
        st = np.asarray(stats_list[k], np.float64)
        a1 = st[t_of_g, p_of_g, 0]
        a2 = st[t_of_g, p_of_g, 1]
        bb = st[t_of_g, p_of_g, 2]
        s3 = st[t_of_g, p_of_g, 3]
        s4 = st[t_of_g, p_of_g, 4]
        s5 = st[t_of_g, p_of_g, 5]
        rows = k * ROWS_PER_CORE + r_of_g
        scr0 = np.floor(a1 / PACK_W)
        scr1 = np.floor(a2 / PACK_W)
        up_lo[rows, b_of_g] = (a1 - PACK_W * scr0) + (NSUF + s3) / 2.0
        dn_hi[rows, b_of_g] = (a2 - PACK_W * scr1) + (NSUF + s4) / 2.0
        n_b[rows, b_of_g] = bb
        screen_ok[rows, b_of_g] = (scr0 + scr1 == 0.0) & (s5 == float(NSUF))

    # ---- reflect-tail blocks 120, 121 (host-side exact counts) -------
    # padded[360000+k] = x[359998-k]; block 120 = x[356999:359999],
    # block 121 = x[353999:356999] as multisets.
    for bidx, sl in ((NBLK, slice(356999, 359999)), (NBLK + 1, slice(353999, 356999))):
        seg = flat[:, sl]
        up_lo[:, bidx] = (seg <= np.float32(-T_A)).sum(1)
        dn_hi[:, bidx] = (seg <= np.float32(T_A)).sum(1)
        n_b[:, bidx] = (seg * seg <= np.float32(T2B)).sum(1)

    # ---- window certificates -----------------------------------------
    w = np.arange(N_WIN)
    cw_lo = up_lo[:, w] + up_lo[:, w + 1]
    cw_hi = dn_hi[:, w] + dn_hi[:, w + 1]
    cw_b = n_b[:, w] + n_b[:, w + 1]
    med_ok = (cw_lo <= 2999 - TIE_SLACK) & (cw_hi >= 3000)
    mad_ok = med_ok & (cw_b <= 2999)                     # => mad_w > T0

    # block b is clear if screen passed and every window feeding its mad_t
    # interpolation (b-1, b, b+1 clamped to [0, 120]) certifies mad > T0.
    win_ok_ext = np.ones((ROWS, N_WIN + 2), bool)
    win_ok_ext[:, 1:N_WIN + 1] = mad_ok
    b = np.arange(NBLK)
    wlo = np.maximum(b - 1, 0)
    whi = np.minimum(b + 1, N_WIN - 1)
    blocks_ok = (screen_ok
                 & win_ok_ext[:, wlo + 1] & win_ok_ext[:, b + 1]
                 & win_ok_ext[:, whi + 1])

    # ---- exact resolution of unclear blocks --------------------------
    # clear rows: no detections -> scores all +0.0 at the first 100
    # non-negative positions (total-order tie-break, see _zero_fill_indices)
    scores = np.zeros((ROWS, TOP_K), np.float32)
    inds = np.empty((ROWS, TOP_K), np.int32)
    for r in range(ROWS):
        inds[r] = _zero_fill_indices(flat[r], (), TOP_K)
    bad_rows = np.nonzero(~blocks_ok.all(axis=1))[0]
    for r in bad_rows:
        xr = flat[r]
        xr_padded = np.pad(xr, (0, MED_K), mode="reflect")
        wcache = {}
        dets = []
        for bb in np.nonzero(~blocks_ok[r])[0]:
            dets.extend(_resolve_block(xr, xr_padded, int(bb), wcache))
        s, i = _assemble_row(xr, dets)
        scores[r] = s
        inds[r] = i
    return scores, inds


# =====================================================================
# Entry point
# =====================================================================
def _spot_check(flat, stats_list, n_checks=12):
    """Verify device counts on a few random blocks; True iff all exact."""
    rng = np.random.default_rng(0)
    for _ in range(n_checks):
        k = int(rng.integers(N_CORES))
        g = int(rng.integers(BLOCKS_PER_CORE))
        tix, p = divmod(g, TILE_P)
        seg = flat[k * ROWS_PER_CORE:(k + 1) * ROWS_PER_CORE].reshape(-1)[
            g * BLK:(g + 1) * BLK]
        want = _block_stats(seg[None, :])[0]
        st = np.asarray(stats_list[k])
        if not np.array_equal(st[tix, p, :6], want[:6]):
            return False
    return True


def kernel(xcorr: np.ndarray):
    flat = np.ascontiguousarray(xcorr, dtype=np.float32).reshape(ROWS, NT)
    try:
        stats_list = _run_device(flat)
        if not _spot_check(flat, stats_list):
            stats_list = compute_stats_numpy(flat)
    except Exception:
        # device unavailable / run failed: exact host fallback
        stats_list = compute_stats_numpy(flat)
    scores, inds = host_postprocess(flat, stats_list)
    return (scores.reshape(2, 3, 16, TOP_K),
            inds.reshape(2, 3, 16, TOP_K).astype(np.int32))


# revision 9
# speedup vs baseline: 1.2985x; 1.1015x over previous
"""Trainium2 Bass kernel for nn_DetectTM (nms_detection).

Reference pipeline per row (96 rows of 360000 f32 samples):
  smax   = sliding window-101 max
  med/mad = lower median / MAD over 121 half-overlapping 6000-sample windows
  mad_t  = bilinear upsample of mad to per-sample resolution
  keep   = (x == smax) & (x > 10*mad_t);  out = top_k(x*keep, 100)

Detection requires x > 10*MAD ~ 6.7 sigma, so detections are (per-dataset)
absent or extremely sparse.  The device kernel is a single-pass *screening*
kernel producing per-block order-statistic certificates; the host proves,
per 3000-sample block, that no sample can pass the threshold — or resolves
the rare uncertified blocks exactly on tiny slices.

Device work per 3000-sample block b (one SBUF partition row):
  Custom 2-stream DVE ops stream TWO elements per cycle (Src0+Src1 on the
  two SBUF read ports), with packed dual counts per accumulator:
    c0 = #{x[0:2*PA] <= -0.1} + 4096 * #{x[0:PA]    >= 5.2}   (op CA2S0)
    c1 = #{x[0:2*PA] <= +0.1} + 4096 * #{x[PA:2*PA] >= 5.2}   (op CA2S1)
    c2 = #{fl32(x*x) <= fl32(0.62^2)} over the whole block     (op CB2)
  ACT covers the block suffix [2*PA, 3000) with three Sign accumulations:
    c3 = sum sign(-0.1 - x),  c4 = sum sign(0.1 - x),  c5 = sum sign(5.2 - x)

Host certificates (counting arguments, sound for any input up to a tie
slack identical in kind to the sign-tie slack of the original design):
  window w spans blocks w, w+1 (6000 samples)
  med_w in (-0.1, 0.1]  if  sum of #{x<=-0.1} upper bounds <= 2999 - slack
                        and sum of #{x<=+0.1} lower bounds >= 3000
  then #{|x - med_w| <= 0.52} <= #{x in [-0.62, 0.62]} <= c2[w]+c2[w+1]
  (monotone RNE rounding: |x| <= 0.62  ==>  fl32(x*x) <= fl32(0.62^2));
  if that is <= 2999 the lower median of |x - med_w| (= mad_w) is > 0.52.
  If every window feeding mad_t over block b certifies mad > 0.52, then
  10*mad_t > 5.2 over the block; zero screen counts + full suffix sign-sum
  prove every x < 5.2; hence keep == False over the whole block.
All-false keep means top_k returns scores 0 at indices 0..99 (jax breaks
value ties by lowest index).  Blocks failing any certificate are resolved
exactly on the host from the raw input (microseconds per block).
"""

import numpy as np

# ---------------------------------------------------------------- constants
N_CORES = 8
ROWS = 96
ROWS_PER_CORE = ROWS // N_CORES        # 12
NT = 360000
BLK = 3000                             # median block / partition stride
NBLK = NT // BLK                       # 120 blocks per row
MED_K = 6000
N_WIN = 121                            # windows per row (incl. reflect tail)
TOP_K = 100
MAXPOOL_K = 101

T_A = 0.1                              # median bracket half-width
T0 = 0.52                              # certified MAD lower bound
T_B = 0.62                             # B-interval threshold (0.1 + 0.52)
T2B = float(np.float32(T_B) * np.float32(T_B))   # squared, f32-exact
T_S = 10.0 * T0                        # 5.2 screening level
PACK_W = 4096.0                        # count packing weight (exact in fp32)
PA = 1014                              # per-half A-op extent (DVE/ACT balance)
NSUF = BLK - 2 * PA                    # ACT suffix elems (972)
HB = BLK // 2                          # CB2 half extent (1500)
TIE_SLACK = 32                         # sign-tie budget on the -0.1 cert

BLOCKS_PER_CORE = ROWS_PER_CORE * NBLK       # 1440
TILE_P = 128
N_TILES = (BLOCKS_PER_CORE + TILE_P - 1) // TILE_P   # 12 (last has 32)
ST_C = 8                               # stats columns per block

_NEG = np.float32(np.finfo(np.float32).min)

# =====================================================================
# Device kernel construction (lazy, cached)
# =====================================================================
_NC_CACHE = {}


def _register_ops():
    """Register the three custom fused 2-stream DVE counting ops.

    Each streams Src0 and Src1 (both SBUF read ports) at 1 elem/cycle per
    port — 2 data elements per cycle — and folds the body into one running
    f32 accumulator (exact: all partial sums < 2^24).
      CA2S0: (in0<=c0) + (in1<=c0) + (in0>=c1)*c2 ; accum add
      CA2S1: (in0<=c0) + (in1<=c0) + (in1>=c1)*c2 ; accum add
      CB2:   (in0*in0<=c0) + (in1*in1<=c0)        ; accum add
    """
    from operator import add
    import concourse.dve_ops as dve_ops
    from concourse.dve_ops import DveOp
    from concourse.dve_spec import (
        Spec, Src0, Src1, C0, C1, C2, Zero, sq, _has_src1, lower)
    from concourse.dve_uop import DveOpSpec

    def _ref_ca2s0(in0, in1, c0, c1, c2):
        out = ((in0 <= c0) + (in1 <= c0) + (in0 >= c1) * c2).astype(np.float32)
        return out, out.reshape(out.shape[0], -1).sum(axis=-1, keepdims=True)

    def _ref_ca2s1(in0, in1, c0, c1, c2):
        out = ((in0 <= c0) + (in1 <= c0) + (in1 >= c1) * c2).astype(np.float32)
        return out, out.reshape(out.shape[0], -1).sum(axis=-1, keepdims=True)

    def _ref_cb2(in0, in1, c0, c1, c2):
        out = ((in0 * in0 <= c0) + (in1 * in1 <= c0)).astype(np.float32)
        return out, out.reshape(out.shape[0], -1).sum(axis=-1, keepdims=True)

    defs = [
        ("CA2S0_ANT",
         ((Src0 <= C0) + (Src1 <= C0)) + (Src0 >= C1) * C2, _ref_ca2s0),
        ("CA2S1_ANT",
         ((Src0 <= C0) + (Src1 <= C0)) + (Src1 >= C1) * C2, _ref_ca2s1),
        ("CB2_ANT",
         (sq(Src0) <= C0) + (sq(Src1) <= C0), _ref_cb2),
    ]
    have = {op.name: op for op in dve_ops.OPS}
    out = []
    for name, body, ref in defs:
        if name in have:
            out.append(have[name])
            continue
        op = DveOp(
            name,
            Spec(body=body, accum=add, accum_init=Zero, reference=ref),
            subdim=False,
            uops_sha={},
        )
        dve_ops.OPS.append(op)
        dve_ops.CUSTOM_DVE_SPECS[op.name] = op.spec
        dve_ops._SUB_OPCODE_FOR_NAME[op.name] = (
            dve_ops._CUSTOM_DVE_ROW_BASE + len(dve_ops.OPS) - 1)
        for ver in ("v3",):
            sha = DveOpSpec(
                name=op.name,
                opcode=dve_ops.get_dve_sub_opcode(op.name),
                uops=lower(op.spec, ver=ver),
                rd1_en=_has_src1(op.spec),
            ).sha(ver)
            op.uops_sha[ver] = sha
        out.append(op)
    return out


def _build_nc():
    import concourse.bacc as bacc
    import concourse.tile as tile
    from concourse import mybir

    ca2s0, ca2s1, cb2 = _register_ops()

    nc = bacc.Bacc("TRN2")
    x_in = nc.dram_tensor("x", [BLOCKS_PER_CORE * BLK], mybir.dt.float32,
                          kind="ExternalInput")
    consts_in = nc.dram_tensor("consts", [TILE_P * 3], mybir.dt.float32,
                               kind="ExternalInput")
    st_out = nc.dram_tensor("stats", [N_TILES, TILE_P, ST_C], mybir.dt.float32,
                            kind="ExternalOutput")

    with tile.TileContext(nc) as tc:
        with (
            tc.tile_pool(name="xtiles", bufs=6) as xpool,
            tc.tile_pool(name="scr", bufs=2) as scrpool,
            tc.tile_pool(name="scr2", bufs=2) as scr2pool,
            tc.tile_pool(name="stats", bufs=1) as stpool,
            tc.tile_pool(name="consts", bufs=1) as cpool,
        ):
            # ACT bias constants arrive by DMA (overlaps the first x tile;
            # avoids gpsimd memsets + an all-engine barrier in the preamble)
            ct = cpool.tile([TILE_P, 3], mybir.dt.float32)
            nc.sync.dma_start(
                out=ct, in_=consts_in.rearrange("(p c) -> p c", p=TILE_P))
            st = stpool.tile([TILE_P, N_TILES * ST_C], mybir.dt.float32)
            for tix in range(N_TILES):
                p0 = tix * TILE_P
                pt = min(TILE_P, BLOCKS_PER_CORE - p0)
                xt = xpool.tile([TILE_P, BLK], mybir.dt.float32)
                src = x_in[p0 * BLK:(p0 + pt) * BLK].rearrange(
                    "(p f) -> p f", p=pt)
                if tix == 0:
                    # split the first transfer so compute starts sooner
                    nc.sync.dma_start(out=xt[:pt, :2 * PA],
                                      in_=src[:, :2 * PA])
                    nc.sync.dma_start(out=xt[:pt, 2 * PA:],
                                      in_=src[:, 2 * PA:])
                else:
                    nc.sync.dma_start(out=xt[:pt], in_=src)
                scr = scrpool.tile([TILE_P, 2 * PA + HB], mybir.dt.float32)
                scr2 = scr2pool.tile([TILE_P, 3 * NSUF], mybir.dt.float32)
                c = st[:, tix * ST_C:(tix + 1) * ST_C]
                # DVE: 2-stream packed counting over [0, 2*PA) + full-block B
                nc.vector._custom_dve(
                    ca2s0, out=scr[:pt, :PA],
                    in0=xt[:pt, :PA], in1=xt[:pt, PA:2 * PA],
                    s0=-T_A, s1=T_S, imm2=PACK_W,
                    accum_out=c[:pt, 0:1])
                nc.vector._custom_dve(
                    ca2s1, out=scr[:pt, PA:2 * PA],
                    in0=xt[:pt, :PA], in1=xt[:pt, PA:2 * PA],
                    s0=T_A, s1=T_S, imm2=PACK_W,
                    accum_out=c[:pt, 1:2])
                nc.vector._custom_dve(
                    cb2, out=scr[:pt, 2 * PA:],
                    in0=xt[:pt, :HB], in1=xt[:pt, HB:],
                    s0=T2B,
                    accum_out=c[:pt, 2:3])
                # ACT: sign-sum suffixes for both A thresholds + screen
                for col in (3, 4, 5):
                    nc.scalar.activation(
                        out=scr2[:pt, (col - 3) * NSUF:(col - 2) * NSUF],
                        in_=xt[:pt, 2 * PA:],
                        func=mybir.ActivationFunctionType.Sign,
                        bias=ct[:pt, col - 3:col - 2], scale=-1.0,
                        accum_out=c[:pt, col:col + 1])
            nc.sync.dma_start(
                out=st_out.rearrange("t p c -> p t c"),
                in_=st.rearrange("p (t c) -> p t c", t=N_TILES))
    nc.finalize()
    return nc


def _get_nc():
    if "nc" not in _NC_CACHE:
        _NC_CACHE["nc"] = _build_nc()
    return _NC_CACHE["nc"]


def _consts_arr():
    return np.tile(np.array([[-T_A, T_A, T_S]], np.float32),
                   (TILE_P, 1)).reshape(-1)


def _in_maps(flat):
    in_maps = []
    for k in range(N_CORES):
        shard = np.ascontiguousarray(
            flat[k * ROWS_PER_CORE:(k + 1) * ROWS_PER_CORE]).reshape(-1)
        in_maps.append({"x": shard, "consts": _consts_arr()})
    return in_maps


def _run_device(flat):
    """flat: [96, NT] f32 -> per-core list of stats [N_TILES, TILE_P, ST_C]."""
    from concourse.bass_utils import run_bass_kernel_spmd
    nc = _get_nc()
    res = run_bass_kernel_spmd(nc, _in_maps(flat), core_ids=list(range(N_CORES)))
    return [r["stats"] for r in res.results]


# =====================================================================
# Host-side emulation of the device stats (for testing / fallback)
# =====================================================================
def _block_stats(blocks):
    """Device per-block stats for blocks: [n, BLK] f32 -> [n, ST_C] f32."""
    h0, h1 = blocks[:, :PA], blocks[:, PA:2 * PA]
    suf = blocks[:, 2 * PA:]
    q0, q1 = blocks[:, :HB], blocks[:, HB:]
    n = blocks.shape[0]
    st = np.zeros((n, ST_C), np.float32)
    st[:, 0] = ((h0 <= np.float32(-T_A)).sum(1) + (h1 <= np.float32(-T_A)).sum(1)
                + PACK_W * (h0 >= np.float32(T_S)).sum(1))
    st[:, 1] = ((h0 <= np.float32(T_A)).sum(1) + (h1 <= np.float32(T_A)).sum(1)
                + PACK_W * (h1 >= np.float32(T_S)).sum(1))
    st[:, 2] = ((q0 * q0 <= np.float32(T2B)).sum(1)
                + (q1 * q1 <= np.float32(T2B)).sum(1))
    st[:, 3] = np.sign(np.float32(-T_A) - suf).sum(1)
    st[:, 4] = np.sign(np.float32(T_A) - suf).sum(1)
    st[:, 5] = np.sign(np.float32(T_S) - suf).sum(1)
    return st


def compute_stats_numpy(flat):
    """Exactly what the device computes, in numpy. flat: [96, NT] f32."""
    out = []
    for k in range(N_CORES):
        shard = flat[k * ROWS_PER_CORE:(k + 1) * ROWS_PER_CORE].reshape(-1)
        st = np.zeros((N_TILES, TILE_P, ST_C), np.float32)
        cols = _block_stats(shard.reshape(BLOCKS_PER_CORE, BLK))
        for tix in range(N_TILES):
            p0 = tix * TILE_P
            pt = min(TILE_P, BLOCKS_PER_CORE - p0)
            st[tix, :pt] = cols[p0:p0 + pt]
        out.append(st)
    return out


# =====================================================================
# Host-side post-processing
# =====================================================================
def _window_slice(xr_padded, w):
    return xr_padded[w * BLK:(w + 2) * BLK]


def _med_mad_window(xr_padded, w, cache):
    got = cache.get(w)
    if got is not None:
        return got
    vals = _window_slice(xr_padded, w)
    mid = (MED_K - 1) // 2
    med = np.partition(vals, mid)[mid]
    mad = np.partition(np.abs(vals - med), mid)[mid]
    cache[w] = (np.float32(med), np.float32(mad))
    return cache[w]


def _resolve_block(xr, xr_padded, b, wcache):
    """Exact keep-mask detections for block b of one row. Returns [(idx,val)]"""
    lo, hi = b * BLK, (b + 1) * BLK
    # sliding 101-max around this block, -inf padded at row edges
    seg = np.full(BLK + 100, _NEG, np.float32)
    s0, s1 = max(lo - 50, 0), min(hi + 50, NT)
    seg[s0 - (lo - 50):s0 - (lo - 50) + (s1 - s0)] = xr[s0:s1]
    from numpy.lib.stride_tricks import sliding_window_view
    smax = sliding_window_view(seg, MAXPOOL_K).max(axis=-1)      # [BLK]

    i = np.arange(lo, hi, dtype=np.float32)
    pos = (i + np.float32(0.5)) / np.float32(BLK) - np.float32(0.5)
    pos = np.maximum(pos, np.float32(0.0))
    x0 = np.minimum(np.floor(pos).astype(np.int32), N_WIN - 1)
    x1 = np.minimum(x0 + 1, N_WIN - 1)
    w = pos - x0.astype(np.float32)
    mad_by_w = np.zeros(N_WIN, np.float32)
    for ww in np.unique(np.concatenate([x0, x1])):
        mad_by_w[ww] = _med_mad_window(xr_padded, int(ww), wcache)[1]
    mad0 = mad_by_w[x0]
    mad1 = mad_by_w[x1]
    mad_t = (mad0 * (np.float32(1.0) - w) + mad1 * w).astype(np.float32)

    xb = xr[lo:hi]
    keep = (xb == smax) & (xb > np.float32(10.0) * mad_t)
    idx = np.nonzero(keep)[0]
    return [(int(lo + j), np.float32(xb[j])) for j in idx]


def _zero_fill_indices(xr, det_pos, k):
    """Lowest k indices of +0.0 entries of masked = x*keep.

    jax.lax.top_k uses the IEEE total order, so +0.0 (x >= 0, keep False)
    ranks above -0.0 (x < 0); ties break by lowest index."""
    scan = 1024
    while True:
        idx = np.nonzero(~np.signbit(xr[:scan]))[0]
        if det_pos:
            idx = idx[~np.isin(idx, list(det_pos))]
        if len(idx) >= k or scan >= NT:
            break
        scan *= 8
    if len(idx) >= k:
        return idx[:k].tolist()
    fills = idx.tolist()           # pathological: < k non-negatives in row
    j = 0
    have = set(fills)
    while len(fills) < k:
        if j not in det_pos and j not in have:
            fills.append(j)
        j += 1
    return fills


def _assemble_row(xr, dets):
    """jax.lax.top_k(masked, 100) given the exact sparse detection list."""
    if not dets:
        fills = _zero_fill_indices(xr, (), TOP_K)
        return (np.array([xr[j] * np.float32(0.0) for j in fills], np.float32),
                np.array(fills, np.int32))
    dets = sorted(dets, key=lambda t: (-t[1], t[0]))
    if len(dets) >= TOP_K:
        top = dets[:TOP_K]
        return (np.array([v for _, v in top], np.float32),
                np.array([i for i, _ in top], np.int32))
    det_pos = set(i for i, _ in dets)
    fills = _zero_fill_indices(xr, det_pos, TOP_K - len(dets))
    vals = [v for _, v in dets] + [np.float32(xr[j] * np.float32(0.0)) for j in fills]
    idxs = [i for i, _ in dets] + fills
    return np.array(vals, np.float32), np.array(idxs, np.int32)


def host_postprocess(flat, stats_list):
    """flat: [96, NT] f32; stats_list: per-core [N_TILES, TILE_P, ST_C].
    Returns (scores [96,100] f32, inds [96,100] i32)."""
    # ---- decode per-block certificate quantities ---------------------
    # up_lo: upper bound of #{x <= -0.1}  (DVE prefix exact + sign suffix;
    #        the sign estimate errs by 1/2 per exact f32 tie — TIE_SLACK
    #        at the window level absorbs it, as in the original design)
    # dn_hi: lower bound of #{x <= +0.1}  (ties only increase the count)
    # n_b:   exact #{fl32(x^2) <= fl32(0.62^2)}  (superset of |x| <= 0.62)
    # screen_ok: certified no x >= 5.2 in the block
    up_lo = np.zeros((ROWS, NBLK + 2), np.float64)
    dn_hi = np.zeros((ROWS, NBLK + 2), np.float64)
    n_b = np.zeros((ROWS, NBLK + 2), np.float64)
    screen_ok = np.zeros((ROWS, NBLK), bool)

    g = np.arange(BLOCKS_PER_CORE)
    t_of_g, p_of_g = divmod(g, TILE_P)
    r_of_g, b_of_g = divmod(g, NBLK)
    for k in range(N_CORES):
        st = np.asarray(stats_list[k], np.float64)
        a1 = st[t_of_g, p_of_g, 0]
        a2 = st[t_of_g, p_of_g, 1]
        bb = st[t_of_g, p_of_g, 2]
        s3 = st[t_of_g, p_of_g, 3]
        s4 = st[t_of_g, p_of_g, 4]
        s5 = st[t_of_g, p_of_g, 5]
        rows = k * ROWS_PER_CORE + r_of_g
        scr0 = np.floor(a1 / PACK_W)
        scr1 = np.floor(a2 / PACK_W)
        up_lo[rows, b_of_g] = (a1 - PACK_W * scr0) + (NSUF + s3) / 2.0
        dn_hi[rows, b_of_g] = (a2 - PACK_W * scr1) + (NSUF + s4) / 2.0
        n_b[rows, b_of_g] = bb
        screen_ok[rows, b_of_g] = (scr0 + scr1 == 0.0) & (s5 == float(NSUF))

    # ---- reflect-tail blocks 120, 121 (host-side exact counts) -------
    # padded[360000+k] = x[359998-k]; block 120 = x[356999:359999],
    # block 121 = x[353999:356999] as multisets.
    for bidx, sl in ((NBLK, slice(356999, 359999)), (NBLK + 1, slice(353999, 356999))):
        seg = flat[:, sl]
        up_lo[:, bidx] = (seg <= np.float32(-T_A)).sum(1)
        dn_hi[:, bidx] = (seg <= np.float32(T_A)).sum(1)
        n_b[:, bidx] = (seg * seg <= np.float32(T2B)).sum(1)

    # ---- window certificates -----------------------------------------
    w = np.arange(N_WIN)
    cw_lo = up_lo[:, w] + up_lo[:, w + 1]
    cw_hi = dn_hi[:, w] + dn_hi[:, w + 1]
    cw_b = n_b[:, w] + n_b[:, w + 1]
    med_ok = (cw_lo <= 2999 - TIE_SLACK) & (cw_hi >= 3000)
    mad_ok = med_ok & (cw_b <= 2999)                     # => mad_w > T0

    # block b is clear if screen passed and every window feeding its mad_t
    # interpolation (b-1, b, b+1 clamped to [0, 120]) certifies mad > T0.
    win_ok_ext = np.ones((ROWS, N_WIN + 2), bool)
    win_ok_ext[:, 1:N_WIN + 1] = mad_ok
    b = np.arange(NBLK)
    wlo = np.maximum(b - 1, 0)
    whi = np.minimum(b + 1, N_WIN - 1)
    blocks_ok = (screen_ok
                 & win_ok_ext[:, wlo + 1] & win_ok_ext[:, b + 1]
                 & win_ok_ext[:, whi + 1])

    # ---- exact resolution of unclear blocks --------------------------
    # clear rows: no detections -> scores all +0.0 at the first 100
    # non-negative positions (total-order tie-break, see _zero_fill_indices)
    scores = np.zeros((ROWS, TOP_K), np.float32)
    inds = np.empty((ROWS, TOP_K), np.int32)
    for r in range(ROWS):
        inds[r] = _zero_fill_indices(flat[r], (), TOP_K)
    bad_rows = np.nonzero(~blocks_ok.all(axis=1))[0]
    for r in bad_rows:
        xr = flat[r]
        xr_padded = np.pad(xr, (0, MED_K), mode="reflect")
        wcache = {}
        dets = []
        for bb in np.nonzero(~blocks_ok[r])[0]:
            dets.extend(_resolve_block(xr, xr_padded, int(bb), wcache))
        s, i = _assemble_row(xr, dets)
        scores[r] = s
        inds[r] = i
    return scores, inds


# =====================================================================
# Entry point
# =====================================================================
def _spot_check(flat, stats_list, n_checks=12):
    """Verify device counts on a few random blocks; True iff all exact."""
    rng = np.random.default_rng(0)
    for _ in range(n_checks):
        k = int(rng.integers(N_CORES))
        g = int(rng.integers(BLOCKS_PER_CORE))
        tix, p = divmod(g, TILE_P)
        seg = flat[k * ROWS_PER_CORE:(k + 1) * ROWS_PER_CORE].reshape(-1)[
            g * BLK:(g + 1) * BLK]
        want = _block_stats(seg[None, :])[0]
        st = np.asarray(stats_list[k])
        if not np.array_equal(st[tix, p, :6], want[:6]):
            return False
    return True


def kernel(xcorr: np.ndarray):
    flat = np.ascontiguousarray(xcorr, dtype=np.float32).reshape(ROWS, NT)
    try:
        stats_list = _run_device(flat)
        if not _spot_check(flat, stats_list):
            stats_list = compute_stats_numpy(flat)
    except Exception:
        # device unavailable / run failed: exact host fallback
        stats_list = compute_stats_numpy(flat)
    scores, inds = host_postprocess(flat, stats_list)
    return (scores.reshape(2, 3, 16, TOP_K),
            inds.reshape(2, 3, 16, TOP_K).astype(np.int32))
